# revision 33
# baseline (speedup 1.0000x reference)
"""Causal self-attention (B=4, T=2048, C=1024, H=16, D=64) on 8 TRN2 NeuronCores.

Sharding: 2D (batch x head-group). Core c handles batch b = c//2 and head
group g = c%2 (heads 8g..8g+7).  Host pre-transposes all inputs so the
device kernel needs no on-chip transposes:
  - xT  [C, T]    : x[b].T
  - wqT/wkT/wvT [C, 512] : w_qkv row-slices for this head group, transposed
  - woT [512, C]  : w_proj column-slice, transposed
Each core computes a partial projected output yT [C, T] for its batch
(contribution of its 8 heads); the host sums the two head-group partials
per batch and transposes back.

Device flow per core (all matmuls are PE `out = lhsT.T @ rhs`, fp32r/TF32):
  A. v = x @ wv^T in natural [tok, feat] layout, resident in SBUF with a
     ones column appended per head (softmax denominator comes free from
     the PV matmul's row 64).
  B. Per head-pair hp: project qT/kT for just these 128 features, then
     attention: scores sT[k,q] via row-tiled concurrent matmuls (two heads
     share the 128-partition dim, K=64 each), exp on ScalarE (1/8 scale
     fused), causal = skip above-diagonal k-tiles + mask diagonal tiles on
     VectorE, PV matmul accumulates oT [65, 512].  The q/k projection of
     hp+1 gives the PE independent work while ACT runs hp's exps (keeps
     the HAM clock-gate at 2.4 GHz).  Normalization: 1/r via exp(-ln(r))
     on ScalarE, partition-broadcast via ones-matmul, multiply on DVE.
  C. yT = woT.T @ o_pack accumulated over the 4 head pairs.
"""

import os
from contextlib import ExitStack

import numpy as np
import ml_dtypes

import concourse.bass as bass
import concourse.bacc as bacc
import concourse.mybir as mybir
import concourse.tile as tile
from concourse.bass_utils import run_bass_kernel_spmd, checkenv

B, T, C = 4, 2048, 1024
H, D = 16, 64
NCORES = 8
F = 512                    # qkv features per matrix per core (8 heads x 64)
CT = C // 128              # 8 contraction tiles
TT = T // 128              # 16 token tiles
QB = T // 512              # 4 query blocks of 512
HP = 4                     # head pairs per core

DTYPE_MODE = os.environ.get("KERNEL_DTYPE", "f32r")  # f32 | f32r | bf16

_F32 = mybir.dt.float32
_EXP = mybir.ActivationFunctionType.Exp
_LN = mybir.ActivationFunctionType.Ln

_cache = {}


def _build_nc():
    dt_store = {"f32": _F32, "f32r": mybir.dt.float32r,
                "bf16": mybir.dt.bfloat16}[DTYPE_MODE]

    nc = bacc.Bacc("TRN2", target_bir_lowering=False, debug=False,
                   num_devices=NCORES)

    xT = nc.dram_tensor("xT", [C, T], dt_store, kind="ExternalInput").ap()
    wqT = nc.dram_tensor("wqT", [C, F], dt_store, kind="ExternalInput").ap()
    wkT = nc.dram_tensor("wkT", [C, F], dt_store, kind="ExternalInput").ap()
    wvT = nc.dram_tensor("wvT", [C, F], dt_store, kind="ExternalInput").ap()
    woT = nc.dram_tensor("woT", [F, C], dt_store, kind="ExternalInput").ap()
    maskd = nc.dram_tensor("mask", [128, 4, 512], dt_store,
                           kind="ExternalInput").ap()
    onesd = nc.dram_tensor("ones", [128, 128], dt_store,
                           kind="ExternalInput").ap()
    yT = nc.dram_tensor("yT", [C, T], _F32, kind="ExternalOutput").ap()

    xT_r = xT.rearrange("(ct p) t -> p ct t", p=128)

    with tile.TileContext(nc) as tc, ExitStack() as top:
        opool = top.enter_context(tc.tile_pool(name="opack", bufs=1))
        onepool = top.enter_context(tc.tile_pool(name="ones", bufs=1))
        ab = top.enter_context(ExitStack())  # pools freed before phase C
        xpool = ab.enter_context(tc.tile_pool(name="xin", bufs=2))
        wqk_pool = ab.enter_context(tc.tile_pool(name="wqk", bufs=1))
        vpool = ab.enter_context(tc.tile_pool(name="vfull", bufs=1))
        mpool = ab.enter_context(tc.tile_pool(name="msk", bufs=1))
        qkpool = ab.enter_context(tc.tile_pool(name="qk", bufs=2))
        epool = ab.enter_context(tc.tile_pool(
            name="exp", bufs=4 if dt_store == mybir.dt.bfloat16 else 2))
        osbpool = ab.enter_context(tc.tile_pool(name="osb", bufs=1))
        rcpool = ab.enter_context(tc.tile_pool(name="rc", bufs=1))
        qkvps = ab.enter_context(tc.tile_pool(name="qkv_ps", bufs=2,
                                              space="PSUM"))

        wq_sb = wqk_pool.tile([128, CT, F], dt_store)
        wk_sb = wqk_pool.tile([128, CT, F], dt_store)
        nc.sync.dma_start(out=wq_sb[:],
                          in_=wqT.rearrange("(ct p) f -> p ct f", p=128))
        nc.sync.dma_start(out=wk_sb[:],
                          in_=wkT.rearrange("(ct p) f -> p ct f", p=128))
        mask_sb = mpool.tile([128, 4, 512], dt_store)
        nc.sync.dma_start(out=mask_sb[:], in_=maskd[:])
        ones_sb = onepool.tile([128, 128], dt_store)
        nc.sync.dma_start(out=ones_sb[:], in_=onesd)

        v_full = vpool.tile([128, TT, 8, D + 1], dt_store)
        nc.sync.dma_start(out=v_full[:, :, :, D:D + 1], in_=onesd[:, 0:128])
        o_pack = opool.tile([128, HP, T], dt_store)

        # ---------------- Phase A: v projection (natural layout) --------
        with tc.tile_pool(name="wv", bufs=1) as wvpool:
            wv_sb = wvpool.tile([128, CT, F], dt_store)
            nc.sync.dma_start(out=wv_sb[:],
                              in_=wvT.rearrange("(ct p) f -> p ct f", p=128))
            for tq in range(4):
                x_q = xpool.tile([128, CT, 512], dt_store, tag="x")
                nc.sync.dma_start(out=x_q[:],
                                  in_=xT_r[:, :, tq * 512:(tq + 1) * 512])
                for tl in range(4):
                    ps = qkvps.tile([128, 512], _F32, tag="ps")
                    for ct in range(CT):
                        nc.tensor.matmul(
                            ps[:], x_q[:, ct, tl * 128:(tl + 1) * 128],
                            wv_sb[:, ct, :],
                            start=(ct == 0), stop=(ct == CT - 1))
                    nc.vector.tensor_copy(
                        v_full[:, tq * 4 + tl, :, 0:D],
                        ps[:].rearrange("p (h d) -> p h d", h=8))

        # ---------- Phase B: per head-pair q/k projection + attention ----
        for hp in range(HP):
            fsl = slice(hp * 128, (hp + 1) * 128)
            q_sb = qkpool.tile([128, T], dt_store, tag="q")
            k_sb = qkpool.tile([128, T], dt_store, tag="k")
            for tq in range(4):
                x_q = xpool.tile([128, CT, 512], dt_store, tag="x")
                nc.sync.dma_start(out=x_q[:],
                                  in_=xT_r[:, :, tq * 512:(tq + 1) * 512])
                tsl = slice(tq * 512, (tq + 1) * 512)
                psq = qkvps.tile([128, 512], _F32, tag="ps")
                psk = qkvps.tile([128, 512], _F32, tag="ps")
                for ct in range(CT):
                    nc.tensor.matmul(psq[:], wq_sb[:, ct, fsl],
                                     x_q[:, ct, :],
                                     start=(ct == 0), stop=(ct == CT - 1))
                for ct in range(CT):
                    nc.tensor.matmul(psk[:], wk_sb[:, ct, fsl],
                                     x_q[:, ct, :],
                                     start=(ct == 0), stop=(ct == CT - 1))
                nc.vector.tensor_copy(q_sb[:, tsl], psq[:])
                nc.vector.tensor_copy(k_sb[:, tsl], psk[:])

            with tc.tile_pool(name=f"at{hp}_sc", bufs=2, space="PSUM") as scps, \
                 tc.tile_pool(name=f"at{hp}_o", bufs=2, space="PSUM") as ops:
                for qb in range(QB):
                    kts = 4 * (qb + 1)
                    oA = ops.tile([D + 1, 512], _F32, tag="o")
                    oB = ops.tile([D + 1, 512], _F32, tag="o")
                    qsl = slice(qb * 512, (qb + 1) * 512)

                    pend = []  # software pipeline: scores(kt) ahead of PV(kt-1)

                    def _pv(kt, e2):
                        nc.tensor.matmul(
                            oA[:], v_full[:, kt, 2 * hp, :], e2[:, 0:512],
                            start=(kt == 0), stop=(kt == kts - 1))
                        nc.tensor.matmul(
                            oB[:], v_full[:, kt, 2 * hp + 1, :],
                            e2[:, 512:1024],
                            start=(kt == 0), stop=(kt == kts - 1))

                    for kt in range(kts):
                        # both heads' score tiles in one 2-bank psum tile;
                        # the row-tiled pair (rows 0-63 / 64-127) runs
                        # CONCURRENTLY on the PE and covers the full array,
                        # which keeps the HAM clock-gate at full speed
                        # (half-array matmuls throttle the PE to 1.2 GHz).
                        ps2 = scps.tile([128, 1024], _F32, tag="s")
                        ksl = slice(kt * 128, (kt + 1) * 128)
                        with tc.tile_critical():
                            nc.tensor.matmul(ps2[:, 0:512], k_sb[0:64, ksl],
                                             q_sb[0:64, qsl],
                                             start=True, stop=True,
                                             tile_position=(0, 0))
                            nc.tensor.matmul(ps2[:, 512:1024],
                                             k_sb[64:128, ksl],
                                             q_sb[64:128, qsl],
                                             start=True, stop=True,
                                             tile_position=(64, 0))
                        e2 = epool.tile([128, 1024], dt_store, tag="e")
                        nc.scalar.activation(e2[:], ps2[:], _EXP, scale=0.125)
                        rel = kt - 4 * qb
                        if rel >= 0:  # diagonal tile: causal mask
                            nc.vector.tensor_mul(e2[:, 0:512], e2[:, 0:512],
                                                 mask_sb[:, rel, :])
                            nc.vector.tensor_mul(e2[:, 512:1024],
                                                 e2[:, 512:1024],
                                                 mask_sb[:, rel, :])
                        pend.append((kt, e2))
                        if len(pend) > 1:
                            _pv(*pend.pop(0))
                    _pv(*pend.pop(0))

                    # normalize: o[d, q] * (1/r)[q];  1/r = exp(-ln(r)) on
                    # ScalarE (DVE reciprocal on a 1-partition row is ~6x
                    # slower), then partition-broadcast via ones-matmul.
                    rc = rcpool.tile([128, 1024], _F32, tag="rc")
                    rcx = rcpool.tile([128, 1024], dt_store, tag="rcx")
                    nc.scalar.activation(rc[64:65, 0:512], oA[64:65, :], _LN)
                    nc.scalar.activation(rc[64:65, 512:1024], oB[64:65, :], _LN)
                    nc.scalar.activation(rcx[64:65, :], rc[64:65, :],
                                         _EXP, scale=-1.0)
                    bc2 = scps.tile([128, 1024], _F32, tag="s")
                    bcA = bc2[:, 0:512]
                    bcB = bc2[:, 512:1024]
                    nc.tensor.matmul(bcA, ones_sb[64:65, 0:128],
                                     rcx[64:65, 0:512],
                                     start=True, stop=True,
                                     tile_position=(64, 0))
                    nc.tensor.matmul(bcB, ones_sb[64:65, 0:128],
                                     rcx[64:65, 512:1024],
                                     start=True, stop=True,
                                     tile_position=(64, 0))
                    # heads live at psum partitions 0-64; head B must land
                    # at o_pack partitions 64-127 -> sbuf->sbuf DMA shift
                    tmp = osbpool.tile([64, 1024], _F32, tag="t")
                    nc.vector.tensor_copy(tmp[0:64, 0:512], oA[0:64, :])
                    nc.vector.tensor_copy(tmp[0:64, 512:1024], oB[0:64, :])
                    ob2 = osbpool.tile([128, 512], _F32, tag="b")
                    nc.sync.dma_start(out=ob2[64:128, :],
                                      in_=tmp[0:64, 512:1024])
                    nc.vector.tensor_mul(o_pack[0:64, hp, qsl],
                                         tmp[0:64, 0:512], bcA[0:64, :])
                    nc.vector.tensor_mul(o_pack[64:128, hp, qsl],
                                         ob2[64:128, :], bcB[64:128, :])

        ab.close()  # free A/B pools (x, weights, v, qk, exp, ...) for phase C

        # ---------------- Phase C: output projection ----------------
        with tc.tile_pool(name="wo", bufs=1) as wopool, \
             tc.tile_pool(name="y_st", bufs=4) as ystpool, \
             tc.tile_pool(name="y_ps", bufs=4, space="PSUM") as yps:
            wo_sb = wopool.tile([128, HP, C], dt_store)
            nc.sync.dma_start(out=wo_sb[:],
                              in_=woT.rearrange("(hp p) o -> p hp o", p=128))
            for ot in range(C // 128):
                for tb in range(QB):
                    ps = yps.tile([128, 512], _F32, tag="y")
                    for hp in range(HP):
                        nc.tensor.matmul(
                            ps[:], wo_sb[:, hp, ot * 128:(ot + 1) * 128],
                            o_pack[:, hp, tb * 512:(tb + 1) * 512],
                            start=(hp == 0), stop=(hp == HP - 1))
                    yst = ystpool.tile([128, 512], _F32, tag="yst")
                    nc.vector.tensor_copy(yst[:], ps[:])
                    nc.sync.dma_start(
                        out=yT[ot * 128:(ot + 1) * 128,
                               tb * 512:(tb + 1) * 512],
                        in_=yst[:])
    # The act-table pass picks the FIRST set containing each function, which
    # makes Exp and Ln thrash between two table sets (~2.7us per swap, per
    # use).  Steer both to the combined natural_log_exp_and_others set by
    # hiding exp/ln from every other set (indices into act_info.json are
    # unchanged, so act_func_set_id stays valid).
    _orig_gat = bacc.get_activation_tables

    def _gat_combined(arch):
        t = _orig_gat(arch)
        for name, fns in t.items():
            if name != "natural_log_exp_and_others":
                fns.discard(mybir.ActivationFunctionType.Exp)
                fns.discard(mybir.ActivationFunctionType.Ln)
        return t

    bacc.get_activation_tables = _gat_combined
    try:
        nc.compile()  # bacc passes: split >1-wait instrs (TRN2 ISA limit)
    finally:
        bacc.get_activation_tables = _orig_gat
    return nc


def _np_store():
    return np.float32 if DTYPE_MODE in ("f32", "f32r") else ml_dtypes.bfloat16


def _round_tf32(a):
    """Round-to-nearest-even onto the TF32 (10-bit mantissa) grid."""
    u = np.ascontiguousarray(a, dtype=np.float32).view(np.uint32)
    r = (u + 0x0FFF + ((u >> 13) & 1)) & np.uint32(0xFFFFE000)
    return r.view(np.float32)


def _prep(a):
    a = np.ascontiguousarray(a, dtype=np.float32)
    if DTYPE_MODE == "f32r":
        return _round_tf32(a)
    return a.astype(_np_store())


def _make_mask():
    kk = np.arange(128)[:, None, None]
    rr = np.arange(4)[None, :, None]
    qq = np.arange(512)[None, None, :]
    return ((rr * 128 + kk) <= qq).astype(_np_store())


LAST_RESULTS = None


def kernel(x, w_qkv, w_proj):
    global LAST_RESULTS
    if "nc" not in _cache:
        _cache["nc"] = _build_nc()
    nc = _cache["nc"]

    mask = _make_mask()
    x = np.asarray(x, dtype=np.float32).reshape(B, T, C)
    w_qkv = np.asarray(w_qkv, dtype=np.float32)
    w_proj = np.asarray(w_proj, dtype=np.float32)

    in_maps = []
    for core in range(NCORES):
        b, g = core // 2, core % 2
        fsl = slice(g * F, (g + 1) * F)
        in_maps.append({
            "xT": _prep(x[b].T),
            "wqT": _prep(w_qkv[0 * C:1 * C][fsl].T),
            "wkT": _prep(w_qkv[1 * C:2 * C][fsl].T),
            "wvT": _prep(w_qkv[2 * C:3 * C][fsl].T),
            "woT": _prep(w_proj[:, fsl].T),
            "mask": mask,
            "ones": np.ones((128, 128), _np_store()),
        })

    LAST_RESULTS = run_bass_kernel_spmd(
        nc, in_maps, list(range(NCORES)), trace=checkenv("BASS_TRACE"))

    y = np.zeros((B, T, C), np.float32)
    for core in range(NCORES):
        b = core // 2
        y[b] += LAST_RESULTS.results[core]["yT"].T
    return y


# revision 34
# speedup vs baseline: 1.3043x; 1.3043x over previous
"""Causal self-attention (B=4, T=2048, C=1024, H=16, D=64) on 8 TRN2 NeuronCores.

Sharding: 2D (batch x head-group). Core c handles batch b = c//2 and head
group g = c%2 (heads 8g..8g+7).  Host pre-transposes all inputs so the
device kernel needs no on-chip transposes:
  - xT  [C, T]    : x[b].T
  - wqT/wkT/wvT [C, 512] : w_qkv row-slices for this head group, transposed
  - woT [512, C]  : w_proj column-slice, transposed
Each core computes a partial projected output yT [C, T] for its batch
(contribution of its 8 heads); the host sums the two head-group partials
per batch and transposes back.

Device flow per core (all matmuls are PE `out = lhsT.T @ rhs`, fp32r/TF32):
  A. v = x @ wv^T in natural [tok, feat] layout, resident in SBUF with a
     ones column appended per head (softmax denominator comes free from
     the PV matmul's row 64).
  B. Per head-pair hp: project qT/kT for just these 128 features, then
     attention: scores sT[k,q] via row-tiled concurrent matmuls (two heads
     share the 128-partition dim, K=64 each), exp on ScalarE (1/8 scale
     fused), causal = skip above-diagonal k-tiles + mask diagonal tiles on
     VectorE, PV matmul accumulates oT [65, 512].  The q/k projection of
     hp+1 gives the PE independent work while ACT runs hp's exps (keeps
     the HAM clock-gate at 2.4 GHz).  Normalization: 1/r via exp(-ln(r))
     on ScalarE, partition-broadcast via ones-matmul, multiply on DVE.
  C. yT = woT.T @ o_pack accumulated over the 4 head pairs.
"""

import os
from contextlib import ExitStack

import numpy as np
import ml_dtypes

import concourse.bass as bass
import concourse.bacc as bacc
import concourse.mybir as mybir
import concourse.tile as tile
from concourse.bass_utils import run_bass_kernel_spmd, checkenv

B, T, C = 4, 2048, 1024
H, D = 16, 64
NCORES = 8
F = 512                    # qkv features per matrix per core (8 heads x 64)
CT = C // 128              # 8 contraction tiles
TT = T // 128              # 16 token tiles
QB = T // 512              # 4 query blocks of 512
HP = 4                     # head pairs per core

DTYPE_MODE = os.environ.get("KERNEL_DTYPE", "f32r")  # f32 | f32r | bf16

_F32 = mybir.dt.float32
_EXP = mybir.ActivationFunctionType.Exp
_LN = mybir.ActivationFunctionType.Ln

_cache = {}


def _build_nc():
    dt_store = {"f32": _F32, "f32r": mybir.dt.float32r,
                "bf16": mybir.dt.bfloat16}[DTYPE_MODE]

    nc = bacc.Bacc("TRN2", target_bir_lowering=False, debug=False,
                   num_devices=NCORES)

    xT = nc.dram_tensor("xT", [C, T], dt_store, kind="ExternalInput").ap()
    wqT = nc.dram_tensor("wqT", [C, F], dt_store, kind="ExternalInput").ap()
    wkT = nc.dram_tensor("wkT", [C, F], dt_store, kind="ExternalInput").ap()
    wvT = nc.dram_tensor("wvT", [C, F], dt_store, kind="ExternalInput").ap()
    woT = nc.dram_tensor("woT", [F, C], dt_store, kind="ExternalInput").ap()
    maskd = nc.dram_tensor("mask", [128, 4, 512], dt_store,
                           kind="ExternalInput").ap()
    onesd = nc.dram_tensor("ones", [128, 128], dt_store,
                           kind="ExternalInput").ap()
    yT = nc.dram_tensor("yT", [C, T], _F32, kind="ExternalOutput").ap()

    xT_r = xT.rearrange("(ct p) t -> p ct t", p=128)

    with tile.TileContext(nc) as tc, ExitStack() as top:
        opool = top.enter_context(tc.tile_pool(name="opack", bufs=1))
        onepool = top.enter_context(tc.tile_pool(name="ones", bufs=1))
        ab = top.enter_context(ExitStack())  # pools freed before phase C
        xpool = ab.enter_context(tc.tile_pool(name="xin", bufs=2))
        wqk_pool = ab.enter_context(tc.tile_pool(name="wqk", bufs=1))
        vpool = ab.enter_context(tc.tile_pool(name="vfull", bufs=1))
        mpool = ab.enter_context(tc.tile_pool(name="msk", bufs=1))
        qkpool = ab.enter_context(tc.tile_pool(name="qk", bufs=2))
        epool = ab.enter_context(tc.tile_pool(
            name="exp", bufs=4 if dt_store == mybir.dt.bfloat16 else 2))
        osbpool = ab.enter_context(tc.tile_pool(name="osb", bufs=1))
        rcpool = ab.enter_context(tc.tile_pool(name="rc", bufs=1))
        qkvps = ab.enter_context(tc.tile_pool(name="qkv_ps", bufs=2,
                                              space="PSUM"))

        wq_sb = wqk_pool.tile([128, CT, F], dt_store)
        wk_sb = wqk_pool.tile([128, CT, F], dt_store)
        nc.sync.dma_start(out=wq_sb[:],
                          in_=wqT.rearrange("(ct p) f -> p ct f", p=128))
        nc.sync.dma_start(out=wk_sb[:],
                          in_=wkT.rearrange("(ct p) f -> p ct f", p=128))
        mask_sb = mpool.tile([128, 4, 512], dt_store)
        nc.sync.dma_start(out=mask_sb[:], in_=maskd[:])
        ones_sb = onepool.tile([128, 128], dt_store)
        nc.sync.dma_start(out=ones_sb[:], in_=onesd)

        v_full = vpool.tile([128, TT, 8, D + 1], dt_store)
        nc.sync.dma_start(out=v_full[:, :, :, D:D + 1], in_=onesd[:, 0:128])
        o_pack = opool.tile([128, HP, T], dt_store)

        # ---------------- Phase A: v projection (natural layout) --------
        with tc.tile_pool(name="wv", bufs=1) as wvpool:
            wv_sb = wvpool.tile([128, CT, F], dt_store)
            nc.sync.dma_start(out=wv_sb[:],
                              in_=wvT.rearrange("(ct p) f -> p ct f", p=128))
            for tq in range(4):
                x_q = xpool.tile([128, CT, 512], dt_store, tag="x")
                nc.sync.dma_start(out=x_q[:],
                                  in_=xT_r[:, :, tq * 512:(tq + 1) * 512])
                for tl in range(4):
                    ps = qkvps.tile([128, 512], _F32, tag="ps")
                    for ct in range(CT):
                        nc.tensor.matmul(
                            ps[:], x_q[:, ct, tl * 128:(tl + 1) * 128],
                            wv_sb[:, ct, :],
                            start=(ct == 0), stop=(ct == CT - 1))
                    nc.vector.tensor_copy(
                        v_full[:, tq * 4 + tl, :, 0:D],
                        ps[:].rearrange("p (h d) -> p h d", h=8))

        # ---------- Phase B: per head-pair q/k projection + attention ----
        for hp in range(HP):
            fsl = slice(hp * 128, (hp + 1) * 128)
            q_sb = qkpool.tile([128, T], dt_store, tag="q")
            k_sb = qkpool.tile([128, T], dt_store, tag="k")
            for tq in range(4):
                x_q = xpool.tile([128, CT, 512], dt_store, tag="x")
                nc.sync.dma_start(out=x_q[:],
                                  in_=xT_r[:, :, tq * 512:(tq + 1) * 512])
                tsl = slice(tq * 512, (tq + 1) * 512)
                psq = qkvps.tile([128, 512], _F32, tag="ps")
                psk = qkvps.tile([128, 512], _F32, tag="ps")
                for ct in range(CT):
                    nc.tensor.matmul(psq[:], wq_sb[:, ct, fsl],
                                     x_q[:, ct, :],
                                     start=(ct == 0), stop=(ct == CT - 1))
                for ct in range(CT):
                    nc.tensor.matmul(psk[:], wk_sb[:, ct, fsl],
                                     x_q[:, ct, :],
                                     start=(ct == 0), stop=(ct == CT - 1))
                nc.vector.tensor_copy(q_sb[:, tsl], psq[:])
                nc.vector.tensor_copy(k_sb[:, tsl], psk[:])

            with tc.tile_pool(name=f"at{hp}_sc", bufs=2, space="PSUM") as scps, \
                 tc.tile_pool(name=f"at{hp}_o", bufs=2, space="PSUM") as ops:
                for qb in range(QB):
                    kts = 4 * (qb + 1)
                    oA = ops.tile([D + 1, 512], _F32, tag="o")
                    oB = ops.tile([D + 1, 512], _F32, tag="o")
                    qsl = slice(qb * 512, (qb + 1) * 512)

                    pend = []  # software pipeline: scores(kt) ahead of PV(kt-1)

                    def _pv(kt, e2):
                        nc.tensor.matmul(
                            oA[:], v_full[:, kt, 2 * hp, :], e2[:, 0:512],
                            start=(kt == 0), stop=(kt == kts - 1))
                        nc.tensor.matmul(
                            oB[:], v_full[:, kt, 2 * hp + 1, :],
                            e2[:, 512:1024],
                            start=(kt == 0), stop=(kt == kts - 1))

                    for kt in range(kts):
                        # PV of kt-1 first, then the score pair of kt, so
                        # the pair lands adjacent in the PE stream: the
                        # row-tiled pair (rows 0-63 / 64-127) then runs
                        # CONCURRENTLY and covers the full array, keeping
                        # the HAM clock-gate at full speed (half-array
                        # matmuls throttle the PE to 1.2 GHz).
                        if pend:
                            _pv(*pend.pop(0))
                        ps2 = scps.tile([128, 1024], _F32, tag="s")
                        ksl = slice(kt * 128, (kt + 1) * 128)
                        nc.tensor.matmul(ps2[:, 0:512], k_sb[0:64, ksl],
                                         q_sb[0:64, qsl],
                                         start=True, stop=True,
                                         tile_position=(0, 0))
                        nc.tensor.matmul(ps2[:, 512:1024],
                                         k_sb[64:128, ksl],
                                         q_sb[64:128, qsl],
                                         start=True, stop=True,
                                         tile_position=(64, 0))
                        e2 = epool.tile([128, 1024], dt_store, tag="e")
                        nc.scalar.activation(e2[:], ps2[:], _EXP, scale=0.125)
                        rel = kt - 4 * qb
                        if rel >= 0:  # diagonal tile: causal mask
                            nc.vector.tensor_mul(e2[:, 0:512], e2[:, 0:512],
                                                 mask_sb[:, rel, :])
                            nc.vector.tensor_mul(e2[:, 512:1024],
                                                 e2[:, 512:1024],
                                                 mask_sb[:, rel, :])
                        pend.append((kt, e2))
                    _pv(*pend.pop(0))

                    # normalize: o[d, q] * (1/r)[q];  1/r = exp(-ln(r)) on
                    # ScalarE (DVE reciprocal on a 1-partition row is ~6x
                    # slower), then partition-broadcast via ones-matmul.
                    rc = rcpool.tile([128, 1024], _F32, tag="rc")
                    rcx = rcpool.tile([128, 1024], dt_store, tag="rcx")
                    nc.scalar.activation(rc[64:65, 0:512], oA[64:65, :], _LN)
                    nc.scalar.activation(rc[64:65, 512:1024], oB[64:65, :], _LN)
                    nc.scalar.activation(rcx[64:65, :], rc[64:65, :],
                                         _EXP, scale=-1.0)
                    bc2 = scps.tile([128, 1024], _F32, tag="s")
                    bcA = bc2[:, 0:512]
                    bcB = bc2[:, 512:1024]
                    nc.tensor.matmul(bcA, ones_sb[64:65, 0:128],
                                     rcx[64:65, 0:512],
                                     start=True, stop=True,
                                     tile_position=(64, 0))
                    nc.tensor.matmul(bcB, ones_sb[64:65, 0:128],
                                     rcx[64:65, 512:1024],
                                     start=True, stop=True,
                                     tile_position=(64, 0))
                    # heads live at psum partitions 0-64; head B must land
                    # at o_pack partitions 64-127 -> sbuf->sbuf DMA shift
                    tmp = osbpool.tile([64, 1024], _F32, tag="t")
                    nc.vector.tensor_copy(tmp[0:64, 0:512], oA[0:64, :])
                    nc.vector.tensor_copy(tmp[0:64, 512:1024], oB[0:64, :])
                    ob2 = osbpool.tile([128, 512], _F32, tag="b")
                    nc.sync.dma_start(out=ob2[64:128, :],
                                      in_=tmp[0:64, 512:1024])
                    nc.vector.tensor_mul(o_pack[0:64, hp, qsl],
                                         tmp[0:64, 0:512], bcA[0:64, :])
                    nc.vector.tensor_mul(o_pack[64:128, hp, qsl],
                                         ob2[64:128, :], bcB[64:128, :])

        ab.close()  # free A/B pools (x, weights, v, qk, exp, ...) for phase C

        # ---------------- Phase C: output projection ----------------
        with tc.tile_pool(name="wo", bufs=1) as wopool, \
             tc.tile_pool(name="y_st", bufs=4) as ystpool, \
             tc.tile_pool(name="y_ps", bufs=4, space="PSUM") as yps:
            wo_sb = wopool.tile([128, HP, C], dt_store)
            nc.sync.dma_start(out=wo_sb[:],
                              in_=woT.rearrange("(hp p) o -> p hp o", p=128))
            for ot in range(C // 128):
                for tb in range(QB):
                    ps = yps.tile([128, 512], _F32, tag="y")
                    for hp in range(HP):
                        nc.tensor.matmul(
                            ps[:], wo_sb[:, hp, ot * 128:(ot + 1) * 128],
                            o_pack[:, hp, tb * 512:(tb + 1) * 512],
                            start=(hp == 0), stop=(hp == HP - 1))
                    yst = ystpool.tile([128, 512], _F32, tag="yst")
                    nc.vector.tensor_copy(yst[:], ps[:])
                    nc.sync.dma_start(
                        out=yT[ot * 128:(ot + 1) * 128,
                               tb * 512:(tb + 1) * 512],
                        in_=yst[:])
    # The act-table pass picks the FIRST set containing each function, which
    # makes Exp and Ln thrash between two table sets (~2.7us per swap, per
    # use).  Steer both to the combined natural_log_exp_and_others set by
    # hiding exp/ln from every other set (indices into act_info.json are
    # unchanged, so act_func_set_id stays valid).
    _orig_gat = bacc.get_activation_tables

    def _gat_combined(arch):
        t = _orig_gat(arch)
        for name, fns in t.items():
            if name != "natural_log_exp_and_others":
                fns.discard(mybir.ActivationFunctionType.Exp)
                fns.discard(mybir.ActivationFunctionType.Ln)
        return t

    bacc.get_activation_tables = _gat_combined
    try:
        nc.compile()  # bacc passes: split >1-wait instrs (TRN2 ISA limit)
    finally:
        bacc.get_activation_tables = _orig_gat
    return nc


def _np_store():
    return np.float32 if DTYPE_MODE in ("f32", "f32r") else ml_dtypes.bfloat16


def _round_tf32(a):
    """Round-to-nearest-even onto the TF32 (10-bit mantissa) grid."""
    u = np.ascontiguousarray(a, dtype=np.float32).view(np.uint32)
    r = (u + 0x0FFF + ((u >> 13) & 1)) & np.uint32(0xFFFFE000)
    return r.view(np.float32)


def _prep(a):
    a = np.ascontiguousarray(a, dtype=np.float32)
    if DTYPE_MODE == "f32r":
        return _round_tf32(a)
    return a.astype(_np_store())


def _make_mask():
    kk = np.arange(128)[:, None, None]
    rr = np.arange(4)[None, :, None]
    qq = np.arange(512)[None, None, :]
    return ((rr * 128 + kk) <= qq).astype(_np_store())


LAST_RESULTS = None


def kernel(x, w_qkv, w_proj):
    global LAST_RESULTS
    if "nc" not in _cache:
        _cache["nc"] = _build_nc()
    nc = _cache["nc"]

    mask = _make_mask()
    x = np.asarray(x, dtype=np.float32).reshape(B, T, C)
    w_qkv = np.asarray(w_qkv, dtype=np.float32)
    w_proj = np.asarray(w_proj, dtype=np.float32)

    in_maps = []
    for core in range(NCORES):
        b, g = core // 2, core % 2
        fsl = slice(g * F, (g + 1) * F)
        in_maps.append({
            "xT": _prep(x[b].T),
            "wqT": _prep(w_qkv[0 * C:1 * C][fsl].T),
            "wkT": _prep(w_qkv[1 * C:2 * C][fsl].T),
            "wvT": _prep(w_qkv[2 * C:3 * C][fsl].T),
            "woT": _prep(w_proj[:, fsl].T),
            "mask": mask,
            "ones": np.ones((128, 128), _np_store()),
        })

    LAST_RESULTS = run_bass_kernel_spmd(
        nc, in_maps, list(range(NCORES)), trace=checkenv("BASS_TRACE"))

    y = np.zeros((B, T, C), np.float32)
    for core in range(NCORES):
        b = core // 2
        y[b] += LAST_RESULTS.results[core]["yT"].T
    return y


# revision 41
# speedup vs baseline: 1.4594x; 1.1189x over previous
"""Causal self-attention (B=4, T=2048, C=1024, H=16, D=64) on 8 TRN2 NeuronCores.

Sharding: 2D (batch x head-group). Core c handles batch b = c//2 and head
group g = c%2 (heads 8g..8g+7).  Host pre-transposes all inputs so the
device kernel needs no on-chip transposes:
  - xT  [C, T]    : x[b].T
  - wqT/wkT/wvT [C, 512] : w_qkv row-slices for this head group, transposed
  - woT [512, C]  : w_proj column-slice, transposed
Each core computes a partial projected output yT [C, T] for its batch
(contribution of its 8 heads); the host sums the two head-group partials
per batch and transposes back.

Device flow per core (all matmuls are PE `out = lhsT.T @ rhs`, fp32r/TF32):
  A. v = x @ wv^T in natural [tok, feat] layout, resident in SBUF with a
     ones column appended per head (softmax denominator comes free from
     the PV matmul's row 64).
  B. Per head-pair hp: project qT/kT for just these 128 features, then
     attention: scores sT[k,q] via row-tiled concurrent matmuls (two heads
     share the 128-partition dim, K=64 each), exp on ScalarE (1/8 scale
     fused), causal = skip above-diagonal k-tiles + mask diagonal tiles on
     VectorE, PV matmul accumulates oT [65, 512].  The q/k projection of
     hp+1 gives the PE independent work while ACT runs hp's exps (keeps
     the HAM clock-gate at 2.4 GHz).  Normalization: 1/r via exp(-ln(r))
     on ScalarE, partition-broadcast via ones-matmul, multiply on DVE.
  C. yT = woT.T @ o_pack accumulated over the 4 head pairs.
"""

import os
from contextlib import ExitStack

import numpy as np
import ml_dtypes

import concourse.bass as bass
import concourse.bacc as bacc
import concourse.mybir as mybir
import concourse.tile as tile
from concourse.bass_utils import run_bass_kernel_spmd, checkenv

B, T, C = 4, 2048, 1024
H, D = 16, 64
NCORES = 8
F = 512                    # qkv features per matrix per core (8 heads x 64)
CT = C // 128              # 8 contraction tiles
TT = T // 128              # 16 token tiles
QB = T // 512              # 4 query blocks of 512
HP = 4                     # head pairs per core

DTYPE_MODE = os.environ.get("KERNEL_DTYPE", "f32r")  # f32 | f32r | bf16

_F32 = mybir.dt.float32
_EXP = mybir.ActivationFunctionType.Exp
_LN = mybir.ActivationFunctionType.Ln

_cache = {}


def _build_nc():
    dt_store = {"f32": _F32, "f32r": mybir.dt.float32r,
                "bf16": mybir.dt.bfloat16}[DTYPE_MODE]

    nc = bacc.Bacc("TRN2", target_bir_lowering=False, debug=False,
                   num_devices=NCORES)

    xT = nc.dram_tensor("xT", [C, T], dt_store, kind="ExternalInput").ap()
    wqT = nc.dram_tensor("wqT", [C, F], dt_store, kind="ExternalInput").ap()
    wkT = nc.dram_tensor("wkT", [C, F], dt_store, kind="ExternalInput").ap()
    wvT = nc.dram_tensor("wvT", [C, F], dt_store, kind="ExternalInput").ap()
    woT = nc.dram_tensor("woT", [F, C], dt_store, kind="ExternalInput").ap()
    maskd = nc.dram_tensor("mask", [128, 4, 512], dt_store,
                           kind="ExternalInput").ap()
    onesd = nc.dram_tensor("ones", [128, 128], dt_store,
                           kind="ExternalInput").ap()
    yT = nc.dram_tensor("yT", [C, T], _F32, kind="ExternalOutput").ap()

    xT_r = xT.rearrange("(ct p) t -> p ct t", p=128)

    with tile.TileContext(nc) as tc, ExitStack() as top:
        opool = top.enter_context(tc.tile_pool(name="opack", bufs=1))
        onepool = top.enter_context(tc.tile_pool(name="ones", bufs=1))
        ab = top.enter_context(ExitStack())  # pools freed before phase C
        xpool = ab.enter_context(tc.tile_pool(name="xin", bufs=2))
        wqk_pool = ab.enter_context(tc.tile_pool(name="wqk", bufs=1))
        vpool = ab.enter_context(tc.tile_pool(name="vfull", bufs=1))
        mpool = ab.enter_context(tc.tile_pool(name="msk", bufs=1))
        qkpool = ab.enter_context(tc.tile_pool(name="qk", bufs=2))
        epool = ab.enter_context(tc.tile_pool(
            name="exp", bufs=6 if dt_store == mybir.dt.bfloat16 else 2))
        osbpool = ab.enter_context(tc.tile_pool(name="osb", bufs=1))
        rcpool = ab.enter_context(tc.tile_pool(name="rc", bufs=1))
        qkvps = ab.enter_context(tc.tile_pool(name="qkv_ps", bufs=1,
                                              space="PSUM"))

        wq_sb = wqk_pool.tile([128, CT, F], dt_store)
        wk_sb = wqk_pool.tile([128, CT, F], dt_store)
        nc.sync.dma_start(out=wq_sb[:],
                          in_=wqT.rearrange("(ct p) f -> p ct f", p=128))
        nc.sync.dma_start(out=wk_sb[:],
                          in_=wkT.rearrange("(ct p) f -> p ct f", p=128))
        mask_sb = mpool.tile([128, 4, 512], dt_store)
        nc.sync.dma_start(out=mask_sb[:], in_=maskd[:])
        ones_sb = onepool.tile([128, 128], dt_store)
        nc.sync.dma_start(out=ones_sb[:], in_=onesd)

        # v kept as 4 quarter tiles so attention can start as soon as the
        # first quarter's projection lands (tile-granularity deps)
        v_q = [vpool.tile([128, 4, 8, D + 1], dt_store, tag=f"v{i}",
                          name=f"v_q{i}")
               for i in range(4)]
        for i in range(4):
            nc.sync.dma_start(out=v_q[i][:, :, :, D:D + 1], in_=onesd[:, 0:32])
        o_pack = opool.tile([128, HP, T], dt_store)

        def v_tile(kt, h):
            return v_q[kt // 4][:, kt % 4, h, :]

        # ---------------- Phase A: v projection (natural layout) --------
        with tc.tile_pool(name="wv", bufs=1) as wvpool:
            wv_sb = wvpool.tile([128, CT, F], dt_store)
            nc.sync.dma_start(out=wv_sb[:],
                              in_=wvT.rearrange("(ct p) f -> p ct f", p=128))
            for tq in range(4):
                x_q = xpool.tile([128, CT, 512], dt_store, tag="x")
                nc.sync.dma_start(out=x_q[:],
                                  in_=xT_r[:, :, tq * 512:(tq + 1) * 512])
                for tl in range(4):
                    ps = qkvps.tile([128, 512], _F32, tag="ps")
                    for ct in range(CT):
                        nc.tensor.matmul(
                            ps[:], x_q[:, ct, tl * 128:(tl + 1) * 128],
                            wv_sb[:, ct, :],
                            start=(ct == 0), stop=(ct == CT - 1))
                    nc.vector.tensor_copy(
                        v_q[tq][:, tl, :, 0:D],
                        ps[:].rearrange("p (h d) -> p h d", h=8))

        # ---------- Phase B: per head-pair q/k projection + attention ----
        for hp in range(HP):
            fsl = slice(hp * 128, (hp + 1) * 128)
            q_sb = qkpool.tile([128, T], dt_store, tag="q")
            k_sb = qkpool.tile([128, T], dt_store, tag="k")
            for tq in range(4):
                x_q = xpool.tile([128, CT, 512], dt_store, tag="x")
                nc.sync.dma_start(out=x_q[:],
                                  in_=xT_r[:, :, tq * 512:(tq + 1) * 512])
                tsl = slice(tq * 512, (tq + 1) * 512)
                psq = qkvps.tile([128, 512], _F32, tag="ps")
                psk = qkvps.tile([128, 512], _F32, tag="ps")
                for ct in range(CT):
                    nc.tensor.matmul(psq[:], wq_sb[:, ct, fsl],
                                     x_q[:, ct, :],
                                     start=(ct == 0), stop=(ct == CT - 1))
                for ct in range(CT):
                    nc.tensor.matmul(psk[:], wk_sb[:, ct, fsl],
                                     x_q[:, ct, :],
                                     start=(ct == 0), stop=(ct == CT - 1))
                nc.vector.tensor_copy(q_sb[:, tsl], psq[:])
                nc.vector.tensor_copy(k_sb[:, tsl], psk[:])

            with tc.tile_pool(name=f"at{hp}_sc", bufs=2, space="PSUM") as scps, \
                 tc.tile_pool(name=f"at{hp}_o", bufs=3, space="PSUM") as ops:
                for qb in range(QB):
                    kts = 4 * (qb + 1)
                    oA = ops.tile([D + 1, 512], _F32, tag="o")
                    oB = ops.tile([D + 1, 512], _F32, tag="o")
                    qsl = slice(qb * 512, (qb + 1) * 512)

                    pend = []  # software pipeline: scores(kt) ahead of PV(kt-1)

                    def _pv(kt, e2):
                        nc.tensor.matmul(
                            oA[:], v_tile(kt, 2 * hp), e2[:, 0:512],
                            start=(kt == 0), stop=(kt == kts - 1))
                        nc.tensor.matmul(
                            oB[:], v_tile(kt, 2 * hp + 1),
                            e2[:, 512:1024],
                            start=(kt == 0), stop=(kt == kts - 1))

                    for kt in range(kts):
                        # PV of kt-1 first, then the score pair of kt, so
                        # the pair lands adjacent in the PE stream: the
                        # row-tiled pair (rows 0-63 / 64-127) then runs
                        # CONCURRENTLY and covers the full array, keeping
                        # the HAM clock-gate at full speed (half-array
                        # matmuls throttle the PE to 1.2 GHz).
                        if pend:
                            _pv(*pend.pop(0))
                        ps2 = scps.tile([128, 1024], _F32, tag="s")
                        ksl = slice(kt * 128, (kt + 1) * 128)
                        nc.tensor.matmul(ps2[:, 0:512], k_sb[0:64, ksl],
                                         q_sb[0:64, qsl],
                                         start=True, stop=True,
                                         tile_position=(0, 0))
                        nc.tensor.matmul(ps2[:, 512:1024],
                                         k_sb[64:128, ksl],
                                         q_sb[64:128, qsl],
                                         start=True, stop=True,
                                         tile_position=(64, 0))
                        e2 = epool.tile([128, 1024], dt_store, tag="e")
                        nc.scalar.activation(e2[:], ps2[:], _EXP, scale=0.125)
                        rel = kt - 4 * qb
                        if rel >= 0:  # diagonal tile: causal mask
                            nc.vector.tensor_mul(e2[:, 0:512], e2[:, 0:512],
                                                 mask_sb[:, rel, :])
                            nc.vector.tensor_mul(e2[:, 512:1024],
                                                 e2[:, 512:1024],
                                                 mask_sb[:, rel, :])
                        pend.append((kt, e2))
                    _pv(*pend.pop(0))

                    # normalize: o[d, q] * (1/r)[q];  1/r = exp(-ln(r)) on
                    # ScalarE (DVE reciprocal on a 1-partition row is ~6x
                    # slower), then partition-broadcast via ones-matmul.
                    rc = rcpool.tile([128, 1024], _F32, tag="rc")
                    rcx = rcpool.tile([128, 1024], dt_store, tag="rcx")
                    nc.scalar.activation(rc[64:65, 0:512], oA[64:65, :], _LN)
                    nc.scalar.activation(rc[64:65, 512:1024], oB[64:65, :], _LN)
                    nc.scalar.activation(rcx[64:65, :], rc[64:65, :],
                                         _EXP, scale=-1.0)
                    bc2 = scps.tile([128, 1024], _F32, tag="s")
                    bcA = bc2[:, 0:512]
                    bcB = bc2[:, 512:1024]
                    nc.tensor.matmul(bcA, ones_sb[64:65, 0:128],
                                     rcx[64:65, 0:512],
                                     start=True, stop=True,
                                     tile_position=(64, 0))
                    nc.tensor.matmul(bcB, ones_sb[64:65, 0:128],
                                     rcx[64:65, 512:1024],
                                     start=True, stop=True,
                                     tile_position=(64, 0))
                    # heads live at psum partitions 0-64; head B must land
                    # at o_pack partitions 64-127 -> sbuf->sbuf DMA shift
                    tmp = osbpool.tile([64, 1024], _F32, tag="t")
                    nc.vector.tensor_copy(tmp[0:64, 0:512], oA[0:64, :])
                    nc.vector.tensor_copy(tmp[0:64, 512:1024], oB[0:64, :])
                    ob2 = osbpool.tile([128, 512], _F32, tag="b")
                    nc.sync.dma_start(out=ob2[64:128, :],
                                      in_=tmp[0:64, 512:1024])
                    nc.vector.tensor_mul(o_pack[0:64, hp, qsl],
                                         tmp[0:64, 0:512], bcA[0:64, :])
                    nc.vector.tensor_mul(o_pack[64:128, hp, qsl],
                                         ob2[64:128, :], bcB[64:128, :])

        ab.close()  # free A/B pools (x, weights, v, qk, exp, ...) for phase C

        # ---------------- Phase C: output projection ----------------
        with tc.tile_pool(name="wo", bufs=1) as wopool, \
             tc.tile_pool(name="y_st", bufs=4) as ystpool, \
             tc.tile_pool(name="y_ps", bufs=4, space="PSUM") as yps:
            wo_sb = wopool.tile([128, HP, C], dt_store)
            nc.sync.dma_start(out=wo_sb[:],
                              in_=woT.rearrange("(hp p) o -> p hp o", p=128))
            for ot in range(C // 128):
                for tb in range(QB):
                    ps = yps.tile([128, 512], _F32, tag="y")
                    for hp in range(HP):
                        nc.tensor.matmul(
                            ps[:], wo_sb[:, hp, ot * 128:(ot + 1) * 128],
                            o_pack[:, hp, tb * 512:(tb + 1) * 512],
                            start=(hp == 0), stop=(hp == HP - 1))
                    yst = ystpool.tile([128, 512], _F32, tag="yst")
                    nc.vector.tensor_copy(yst[:], ps[:])
                    nc.sync.dma_start(
                        out=yT[ot * 128:(ot + 1) * 128,
                               tb * 512:(tb + 1) * 512],
                        in_=yst[:])
    # The act-table pass picks the FIRST set containing each function, which
    # makes Exp and Ln thrash between two table sets (~2.7us per swap, per
    # use).  Steer both to the combined natural_log_exp_and_others set by
    # hiding exp/ln from every other set (indices into act_info.json are
    # unchanged, so act_func_set_id stays valid).
    _orig_gat = bacc.get_activation_tables

    def _gat_combined(arch):
        t = _orig_gat(arch)
        for name, fns in t.items():
            if name != "natural_log_exp_and_others":
                fns.discard(mybir.ActivationFunctionType.Exp)
                fns.discard(mybir.ActivationFunctionType.Ln)
        return t

    bacc.get_activation_tables = _gat_combined
    try:
        nc.compile()  # bacc passes: split >1-wait instrs (TRN2 ISA limit)
    finally:
        bacc.get_activation_tables = _orig_gat
    return nc


def _np_store():
    return np.float32 if DTYPE_MODE in ("f32", "f32r") else ml_dtypes.bfloat16


def _round_tf32(a):
    """Round-to-nearest-even onto the TF32 (10-bit mantissa) grid."""
    u = np.ascontiguousarray(a, dtype=np.float32).view(np.uint32)
    r = (u + 0x0FFF + ((u >> 13) & 1)) & np.uint32(0xFFFFE000)
    return r.view(np.float32)


def _prep(a):
    a = np.ascontiguousarray(a, dtype=np.float32)
    if DTYPE_MODE == "f32r":
        return _round_tf32(a)
    return a.astype(_np_store())


def _make_mask():
    kk = np.arange(128)[:, None, None]
    rr = np.arange(4)[None, :, None]
    qq = np.arange(512)[None, None, :]
    return ((rr * 128 + kk) <= qq).astype(_np_store())


LAST_RESULTS = None


def kernel(x, w_qkv, w_proj):
    global LAST_RESULTS
    if "nc" not in _cache:
        _cache["nc"] = _build_nc()
    nc = _cache["nc"]

    mask = _make_mask()
    x = np.asarray(x, dtype=np.float32).reshape(B, T, C)
    w_qkv = np.asarray(w_qkv, dtype=np.float32)
    w_proj = np.asarray(w_proj, dtype=np.float32)

    in_maps = []
    for core in range(NCORES):
        b, g = core // 2, core % 2
        fsl = slice(g * F, (g + 1) * F)
        in_maps.append({
            "xT": _prep(x[b].T),
            "wqT": _prep(w_qkv[0 * C:1 * C][fsl].T),
            "wkT": _prep(w_qkv[1 * C:2 * C][fsl].T),
            "wvT": _prep(w_qkv[2 * C:3 * C][fsl].T),
            "woT": _prep(w_proj[:, fsl].T),
            "mask": mask,
            "ones": np.ones((128, 128), _np_store()),
        })

    LAST_RESULTS = run_bass_kernel_spmd(
        nc, in_maps, list(range(NCORES)), trace=checkenv("BASS_TRACE"))

    y = np.zeros((B, T, C), np.float32)
    for core in range(NCORES):
        b = core // 2
        y[b] += LAST_RESULTS.results[core]["yT"].T
    return y


# revision 42
# speedup vs baseline: 1.4720x; 1.0086x over previous
"""Causal self-attention (B=4, T=2048, C=1024, H=16, D=64) on 8 TRN2 NeuronCores.

Sharding: 2D (batch x head-group). Core c handles batch b = c//2 and head
group g = c%2 (heads 8g..8g+7).  Host pre-transposes all inputs so the
device kernel needs no on-chip transposes:
  - xT  [C, T]    : x[b].T
  - wqT/wkT/wvT [C, 512] : w_qkv row-slices for this head group, transposed
  - woT [512, C]  : w_proj column-slice, transposed
Each core computes a partial projected output yT [C, T] for its batch
(contribution of its 8 heads); the host sums the two head-group partials
per batch and transposes back.

Device flow per core (all matmuls are PE `out = lhsT.T @ rhs`, fp32r/TF32):
  A. v = x @ wv^T in natural [tok, feat] layout, resident in SBUF with a
     ones column appended per head (softmax denominator comes free from
     the PV matmul's row 64).
  B. Per head-pair hp: project qT/kT for just these 128 features, then
     attention: scores sT[k,q] via row-tiled concurrent matmuls (two heads
     share the 128-partition dim, K=64 each), exp on ScalarE (1/8 scale
     fused), causal = skip above-diagonal k-tiles + mask diagonal tiles on
     VectorE, PV matmul accumulates oT [65, 512].  The q/k projection of
     hp+1 gives the PE independent work while ACT runs hp's exps (keeps
     the HAM clock-gate at 2.4 GHz).  Normalization: 1/r via exp(-ln(r))
     on ScalarE, partition-broadcast via ones-matmul, multiply on DVE.
  C. yT = woT.T @ o_pack accumulated over the 4 head pairs.
"""

import os
from contextlib import ExitStack

import numpy as np
import ml_dtypes

import concourse.bass as bass
import concourse.bacc as bacc
import concourse.mybir as mybir
import concourse.tile as tile
from concourse.bass_utils import run_bass_kernel_spmd, checkenv

B, T, C = 4, 2048, 1024
H, D = 16, 64
NCORES = 8
F = 512                    # qkv features per matrix per core (8 heads x 64)
CT = C // 128              # 8 contraction tiles
TT = T // 128              # 16 token tiles
QB = T // 512              # 4 query blocks of 512
HP = 4                     # head pairs per core

DTYPE_MODE = os.environ.get("KERNEL_DTYPE", "f32r")  # f32 | f32r | bf16

_F32 = mybir.dt.float32
_EXP = mybir.ActivationFunctionType.Exp
_LN = mybir.ActivationFunctionType.Ln

_cache = {}


def _build_nc():
    dt_store = {"f32": _F32, "f32r": mybir.dt.float32r,
                "bf16": mybir.dt.bfloat16}[DTYPE_MODE]
    bf16 = dt_store == mybir.dt.bfloat16

    nc = bacc.Bacc("TRN2", target_bir_lowering=False, debug=False,
                   num_devices=NCORES)

    xT = nc.dram_tensor("xT", [C, T], dt_store, kind="ExternalInput").ap()
    wqT = nc.dram_tensor("wqT", [C, F], dt_store, kind="ExternalInput").ap()
    wkT = nc.dram_tensor("wkT", [C, F], dt_store, kind="ExternalInput").ap()
    wvT = nc.dram_tensor("wvT", [C, F], dt_store, kind="ExternalInput").ap()
    woT = nc.dram_tensor("woT", [F, C], dt_store, kind="ExternalInput").ap()
    maskd = nc.dram_tensor("mask", [128, 4, 512], dt_store,
                           kind="ExternalInput").ap()
    onesd = nc.dram_tensor("ones", [128, 128], dt_store,
                           kind="ExternalInput").ap()
    yT = nc.dram_tensor("yT", [C, T], _F32, kind="ExternalOutput").ap()

    xT_r = xT.rearrange("(ct p) t -> p ct t", p=128)

    with tile.TileContext(nc) as tc, ExitStack() as top:
        opool = top.enter_context(tc.tile_pool(name="opack", bufs=1))
        onepool = top.enter_context(tc.tile_pool(name="ones", bufs=1))
        ab = top.enter_context(ExitStack())  # pools freed before phase C
        xpool = ab.enter_context(tc.tile_pool(name="xin", bufs=2))
        wqk_pool = ab.enter_context(tc.tile_pool(name="wqk", bufs=1))
        vpool = ab.enter_context(tc.tile_pool(name="vfull", bufs=1))
        mpool = ab.enter_context(tc.tile_pool(name="msk", bufs=1))
        qkpool = ab.enter_context(tc.tile_pool(name="qk",
                                               bufs=2 if bf16 else 1))
        epool = ab.enter_context(tc.tile_pool(name="exp",
                                              bufs=6 if bf16 else 2))
        osbpool = ab.enter_context(tc.tile_pool(name="osb", bufs=1))
        rcpool = ab.enter_context(tc.tile_pool(name="rc", bufs=1))
        # ONE shared [128,1024] (2-bank) psum pool for every paired matmul
        # group (v pairs, q+k pairs, score pairs, 1/r broadcasts): 6 banks,
        # deep enough that the exp stream never waits for a free bank.
        mmps = ab.enter_context(tc.tile_pool(name="mm_ps", bufs=3,
                                             space="PSUM"))
        ops = ab.enter_context(tc.tile_pool(name="o_ps", bufs=2,
                                            space="PSUM"))

        wq_sb = wqk_pool.tile([128, CT, F], dt_store)
        wk_sb = wqk_pool.tile([128, CT, F], dt_store)
        nc.sync.dma_start(out=wq_sb[:],
                          in_=wqT.rearrange("(ct p) f -> p ct f", p=128))
        nc.sync.dma_start(out=wk_sb[:],
                          in_=wkT.rearrange("(ct p) f -> p ct f", p=128))
        mask_sb = mpool.tile([128, 4, 512], dt_store)
        nc.sync.dma_start(out=mask_sb[:], in_=maskd[:])
        ones_sb = onepool.tile([128, 128], dt_store)
        nc.sync.dma_start(out=ones_sb[:], in_=onesd)

        # v kept as 4 quarter tiles so attention can start as soon as the
        # first quarter's projection lands (tile-granularity deps)
        v_q = [vpool.tile([128, 4, 8, D + 1], dt_store, tag=f"v{i}",
                          name=f"v_q{i}")
               for i in range(4)]
        for i in range(4):
            nc.sync.dma_start(out=v_q[i][:, :, :, D:D + 1], in_=onesd[:, 0:32])
        o_pack = opool.tile([128, HP, T], dt_store)

        def v_tile(kt, h):
            return v_q[kt // 4][:, kt % 4, h, :]

        # q/k held as per-quarter tiles, two head-pairs in flight
        q_t = [[None] * 4 for _ in range(HP)]
        k_t = [[None] * 4 for _ in range(HP)]

        def emit_qk_quarter(hp, tq, x_q):
            """Project q and k (features of head-pair hp) for one 512-token
            quarter: both 8-matmul groups land in one 2-bank psum tile."""
            fsl = slice(hp * 128, (hp + 1) * 128)
            qt = qkpool.tile([128, 512], dt_store, tag=f"q{tq}",
                             name=f"q_{hp}_{tq}")
            kt_ = qkpool.tile([128, 512], dt_store, tag=f"k{tq}",
                              name=f"k_{hp}_{tq}")
            ps = mmps.tile([128, 1024], _F32, tag="s", name=f"qkps_{hp}_{tq}")
            for ct in range(CT):
                nc.tensor.matmul(ps[:, 0:512], wq_sb[:, ct, fsl],
                                 x_q[:, ct, :],
                                 start=(ct == 0), stop=(ct == CT - 1))
            for ct in range(CT):
                nc.tensor.matmul(ps[:, 512:1024], wk_sb[:, ct, fsl],
                                 x_q[:, ct, :],
                                 start=(ct == 0), stop=(ct == CT - 1))
            nc.vector.tensor_copy(qt[:], ps[:, 0:512])
            nc.vector.tensor_copy(kt_[:], ps[:, 512:1024])
            q_t[hp][tq] = qt
            k_t[hp][tq] = kt_

        # ---- Phase A: v projection + hp0 q/k (sharing each x quarter) ----
        with tc.tile_pool(name="wv", bufs=1) as wvpool:
            wv_sb = wvpool.tile([128, CT, F], dt_store)
            nc.sync.dma_start(out=wv_sb[:],
                              in_=wvT.rearrange("(ct p) f -> p ct f", p=128))
            for tq in range(4):
                x_q = xpool.tile([128, CT, 512], dt_store, tag="x",
                                 name=f"xA{tq}")
                nc.sync.dma_start(out=x_q[:],
                                  in_=xT_r[:, :, tq * 512:(tq + 1) * 512])
                emit_qk_quarter(0, tq, x_q)
                for half in range(2):
                    ps = mmps.tile([128, 1024], _F32, tag="s",
                                   name=f"vps_{tq}_{half}")
                    for tl2 in range(2):
                        tl = half * 2 + tl2
                        for ct in range(CT):
                            nc.tensor.matmul(
                                ps[:, tl2 * 512:(tl2 + 1) * 512],
                                x_q[:, ct, tl * 128:(tl + 1) * 128],
                                wv_sb[:, ct, :],
                                start=(ct == 0), stop=(ct == CT - 1))
                    for tl2 in range(2):
                        tl = half * 2 + tl2
                        nc.vector.tensor_copy(
                            v_q[tq][:, tl, :, 0:D],
                            ps[:, tl2 * 512:(tl2 + 1) * 512].rearrange(
                                "p (h d) -> p h d", h=8))

        # ---- Phase B: attention per head-pair, next pair's q/k streamed in
        for hp in range(HP):
            for qb in range(QB):
                if hp + 1 < HP:
                    # project the NEXT head-pair's q/k quarter now: gives
                    # the PE full-array independent work while ACT runs
                    # this pair's exps
                    x_q = xpool.tile([128, CT, 512], dt_store, tag="x",
                                     name=f"xB_{hp}_{qb}")
                    nc.sync.dma_start(
                        out=x_q[:], in_=xT_r[:, :, qb * 512:(qb + 1) * 512])
                    emit_qk_quarter(hp + 1, qb, x_q)

                kts = 4 * (qb + 1)
                oA = ops.tile([D + 1, 512], _F32, tag="o", name=f"oA_{hp}_{qb}")
                oB = ops.tile([D + 1, 512], _F32, tag="o", name=f"oB_{hp}_{qb}")
                q_sb = q_t[hp][qb]

                pend = []  # software pipeline: scores(kt) ahead of PV(kt-1)

                def _pv(kt, e2):
                    nc.tensor.matmul(
                        oA[:], v_tile(kt, 2 * hp), e2[:, 0:512],
                        start=(kt == 0), stop=(kt == kts - 1))
                    nc.tensor.matmul(
                        oB[:], v_tile(kt, 2 * hp + 1), e2[:, 512:1024],
                        start=(kt == 0), stop=(kt == kts - 1))

                for kt in range(kts):
                    # PV of kt-1 first, then the score pair of kt, so the
                    # pair lands adjacent in the PE stream: the row-tiled
                    # pair (rows 0-63 / 64-127) runs CONCURRENTLY and
                    # covers the full array, keeping the HAM clock-gate at
                    # full speed (half-array matmuls throttle to 1.2 GHz).
                    if pend:
                        _pv(*pend.pop(0))
                    ps2 = mmps.tile([128, 1024], _F32, tag="s",
                                    name=f"sc_{hp}_{qb}_{kt}")
                    k_sb = k_t[hp][kt // 4]
                    ksl = slice((kt % 4) * 128, (kt % 4) * 128 + 128)
                    nc.tensor.matmul(ps2[:, 0:512], k_sb[0:64, ksl],
                                     q_sb[0:64, :],
                                     start=True, stop=True,
                                     tile_position=(0, 0))
                    nc.tensor.matmul(ps2[:, 512:1024], k_sb[64:128, ksl],
                                     q_sb[64:128, :],
                                     start=True, stop=True,
                                     tile_position=(64, 0))
                    e2 = epool.tile([128, 1024], dt_store, tag="e",
                                    name=f"e_{hp}_{qb}_{kt}")
                    nc.scalar.activation(e2[:], ps2[:], _EXP, scale=0.125)
                    rel = kt - 4 * qb
                    if rel >= 0:  # diagonal tile: causal mask
                        nc.vector.tensor_mul(e2[:, 0:512], e2[:, 0:512],
                                             mask_sb[:, rel, :])
                        nc.vector.tensor_mul(e2[:, 512:1024],
                                             e2[:, 512:1024],
                                             mask_sb[:, rel, :])
                    pend.append((kt, e2))
                _pv(*pend.pop(0))

                # normalize: o[d, q] * (1/r)[q];  1/r = exp(-ln(r)) on
                # ScalarE (DVE reciprocal on a 1-partition row is ~6x
                # slower), then partition-broadcast via ones-matmul.
                rc = rcpool.tile([128, 1024], _F32, tag="rc",
                                 name=f"rc_{hp}_{qb}")
                rcx = rcpool.tile([128, 1024], dt_store, tag="rcx",
                                  name=f"rcx_{hp}_{qb}")
                nc.scalar.activation(rc[64:65, 0:512], oA[64:65, :], _LN)
                nc.scalar.activation(rc[64:65, 512:1024], oB[64:65, :], _LN)
                nc.scalar.activation(rcx[64:65, :], rc[64:65, :],
                                     _EXP, scale=-1.0)
                bc2 = mmps.tile([128, 1024], _F32, tag="s",
                                name=f"bc_{hp}_{qb}")
                bcA = bc2[:, 0:512]
                bcB = bc2[:, 512:1024]
                nc.tensor.matmul(bcA, ones_sb[64:65, 0:128],
                                 rcx[64:65, 0:512],
                                 start=True, stop=True,
                                 tile_position=(64, 0))
                nc.tensor.matmul(bcB, ones_sb[64:65, 0:128],
                                 rcx[64:65, 512:1024],
                                 start=True, stop=True,
                                 tile_position=(64, 0))
                # heads live at psum partitions 0-64; head B must land at
                # o_pack partitions 64-127 -> sbuf->sbuf DMA shift
                qsl = slice(qb * 512, (qb + 1) * 512)
                tmp = osbpool.tile([64, 1024], _F32, tag="t",
                                   name=f"tmp_{hp}_{qb}")
                nc.vector.tensor_copy(tmp[0:64, 0:512], oA[0:64, :])
                nc.vector.tensor_copy(tmp[0:64, 512:1024], oB[0:64, :])
                ob2 = osbpool.tile([128, 512], _F32, tag="b",
                                   name=f"ob2_{hp}_{qb}")
                nc.sync.dma_start(out=ob2[64:128, :], in_=tmp[0:64, 512:1024])
                nc.vector.tensor_mul(o_pack[0:64, hp, qsl],
                                     tmp[0:64, 0:512], bcA[0:64, :])
                nc.vector.tensor_mul(o_pack[64:128, hp, qsl],
                                     ob2[64:128, :], bcB[64:128, :])

        ab.close()  # free A/B pools (x, weights, v, qk, exp, ...) for phase C

        # ---------------- Phase C: output projection ----------------
        with tc.tile_pool(name="wo", bufs=1) as wopool, \
             tc.tile_pool(name="y_st", bufs=4) as ystpool, \
             tc.tile_pool(name="y_ps", bufs=4, space="PSUM") as yps:
            wo_sb = wopool.tile([128, HP, C], dt_store)
            nc.sync.dma_start(out=wo_sb[:],
                              in_=woT.rearrange("(hp p) o -> p hp o", p=128))
            for ot in range(C // 128):
                for tb in range(QB):
                    ps = yps.tile([128, 512], _F32, tag="y")
                    for hp in range(HP):
                        nc.tensor.matmul(
                            ps[:], wo_sb[:, hp, ot * 128:(ot + 1) * 128],
                            o_pack[:, hp, tb * 512:(tb + 1) * 512],
                            start=(hp == 0), stop=(hp == HP - 1))
                    yst = ystpool.tile([128, 512], _F32, tag="yst")
                    nc.vector.tensor_copy(yst[:], ps[:])
                    nc.sync.dma_start(
                        out=yT[ot * 128:(ot + 1) * 128,
                               tb * 512:(tb + 1) * 512],
                        in_=yst[:])
    # The act-table pass picks the FIRST set containing each function, which
    # makes Exp and Ln thrash between two table sets (~2.7us per swap, per
    # use).  Steer both to the combined natural_log_exp_and_others set by
    # hiding exp/ln from every other set (indices into act_info.json are
    # unchanged, so act_func_set_id stays valid).
    _orig_gat = bacc.get_activation_tables

    def _gat_combined(arch):
        t = _orig_gat(arch)
        for name, fns in t.items():
            if name != "natural_log_exp_and_others":
                fns.discard(mybir.ActivationFunctionType.Exp)
                fns.discard(mybir.ActivationFunctionType.Ln)
        return t

    bacc.get_activation_tables = _gat_combined
    try:
        nc.compile()  # bacc passes: split >1-wait instrs (TRN2 ISA limit)
    finally:
        bacc.get_activation_tables = _orig_gat
    return nc


def _np_store():
    return np.float32 if DTYPE_MODE in ("f32", "f32r") else ml_dtypes.bfloat16


def _round_tf32(a):
    """Round-to-nearest-even onto the TF32 (10-bit mantissa) grid."""
    u = np.ascontiguousarray(a, dtype=np.float32).view(np.uint32)
    r = (u + 0x0FFF + ((u >> 13) & 1)) & np.uint32(0xFFFFE000)
    return r.view(np.float32)


def _prep(a):
    a = np.ascontiguousarray(a, dtype=np.float32)
    if DTYPE_MODE == "f32r":
        return _round_tf32(a)
    return a.astype(_np_store())


def _make_mask():
    kk = np.arange(128)[:, None, None]
    rr = np.arange(4)[None, :, None]
    qq = np.arange(512)[None, None, :]
    return ((rr * 128 + kk) <= qq).astype(_np_store())


LAST_RESULTS = None


def kernel(x, w_qkv, w_proj):
    global LAST_RESULTS
    if "nc" not in _cache:
        _cache["nc"] = _build_nc()
    nc = _cache["nc"]

    mask = _make_mask()
    x = np.asarray(x, dtype=np.float32).reshape(B, T, C)
    w_qkv = np.asarray(w_qkv, dtype=np.float32)
    w_proj = np.asarray(w_proj, dtype=np.float32)

    in_maps = []
    for core in range(NCORES):
        b, g = core // 2, core % 2
        fsl = slice(g * F, (g + 1) * F)
        in_maps.append({
            "xT": _prep(x[b].T),
            "wqT": _prep(w_qkv[0 * C:1 * C][fsl].T),
            "wkT": _prep(w_qkv[1 * C:2 * C][fsl].T),
            "wvT": _prep(w_qkv[2 * C:3 * C][fsl].T),
            "woT": _prep(w_proj[:, fsl].T),
            "mask": mask,
            "ones": np.ones((128, 128), _np_store()),
        })

    LAST_RESULTS = run_bass_kernel_spmd(
        nc, in_maps, list(range(NCORES)), trace=checkenv("BASS_TRACE"))

    y = np.zeros((B, T, C), np.float32)
    for core in range(NCORES):
        b = core // 2
        y[b] += LAST_RESULTS.results[core]["yT"].T
    return y


# revision 45
# speedup vs baseline: 1.7606x; 1.1961x over previous
"""Causal self-attention (B=4, T=2048, C=1024, H=16, D=64) on 8 TRN2 NeuronCores.

Sharding: 2D (batch x head-group). Core c handles batch b = c//2 and head
group g = c%2 (heads 8g..8g+7).  Host pre-transposes all inputs so the
device kernel needs no on-chip transposes:
  - xT  [C, T]    : x[b].T
  - wqT/wkT/wvT [C, 512] : w_qkv row-slices for this head group, transposed
  - woT [512, C]  : w_proj column-slice, transposed
Each core computes a partial projected output yT [C, T] for its batch
(contribution of its 8 heads); the host sums the two head-group partials
per batch and transposes back.

Device flow per core (all matmuls are PE `out = lhsT.T @ rhs`, fp32r/TF32):
  A. v = x @ wv^T in natural [tok, feat] layout, resident in SBUF with a
     ones column appended per head (softmax denominator comes free from
     the PV matmul's row 64).
  B. Per head-pair hp: project qT/kT for just these 128 features, then
     attention: scores sT[k,q] via row-tiled concurrent matmuls (two heads
     share the 128-partition dim, K=64 each), exp on ScalarE (1/8 scale
     fused), causal = skip above-diagonal k-tiles + mask diagonal tiles on
     VectorE, PV matmul accumulates oT [65, 512].  The q/k projection of
     hp+1 gives the PE independent work while ACT runs hp's exps (keeps
     the HAM clock-gate at 2.4 GHz).  Normalization: 1/r via exp(-ln(r))
     on ScalarE, partition-broadcast via ones-matmul, multiply on DVE.
  C. yT = woT.T @ o_pack accumulated over the 4 head pairs.
"""

import os
from contextlib import ExitStack

import numpy as np
import ml_dtypes

import concourse.bass as bass
import concourse.bacc as bacc
import concourse.mybir as mybir
import concourse.tile as tile
from concourse.bass_utils import run_bass_kernel_spmd, checkenv

B, T, C = 4, 2048, 1024
H, D = 16, 64
NCORES = 8
F = 512                    # qkv features per matrix per core (8 heads x 64)
CT = C // 128              # 8 contraction tiles
TT = T // 128              # 16 token tiles
QB = T // 512              # 4 query blocks of 512
HP = 4                     # head pairs per core

DTYPE_MODE = os.environ.get("KERNEL_DTYPE", "f32r")  # f32 | f32r | bf16

_F32 = mybir.dt.float32
_EXP = mybir.ActivationFunctionType.Exp
_LN = mybir.ActivationFunctionType.Ln

_cache = {}


def _build_nc():
    dt_store = {"f32": _F32, "f32r": mybir.dt.float32r,
                "bf16": mybir.dt.bfloat16}[DTYPE_MODE]
    bf16 = dt_store == mybir.dt.bfloat16

    nc = bacc.Bacc("TRN2", target_bir_lowering=False, debug=False,
                   num_devices=NCORES)

    xT = nc.dram_tensor("xT", [C, T], dt_store, kind="ExternalInput").ap()
    wqT = nc.dram_tensor("wqT", [C, F], dt_store, kind="ExternalInput").ap()
    wkT = nc.dram_tensor("wkT", [C, F], dt_store, kind="ExternalInput").ap()
    wvT = nc.dram_tensor("wvT", [C, F], dt_store, kind="ExternalInput").ap()
    woT = nc.dram_tensor("woT", [F, C], dt_store, kind="ExternalInput").ap()
    maskd = nc.dram_tensor("mask", [128, 4, 512], dt_store,
                           kind="ExternalInput").ap()
    onesd = nc.dram_tensor("ones", [128, 128], dt_store,
                           kind="ExternalInput").ap()
    yT = nc.dram_tensor("yT", [C, T], _F32, kind="ExternalOutput").ap()

    xT_r = xT.rearrange("(ct p) t -> p ct t", p=128)

    with tile.TileContext(nc) as tc, ExitStack() as top:
        opool = top.enter_context(tc.tile_pool(name="opack", bufs=1))
        onepool = top.enter_context(tc.tile_pool(name="ones", bufs=1))
        ab = top.enter_context(ExitStack())  # pools freed before phase C
        xpool = ab.enter_context(tc.tile_pool(name="xin", bufs=2))
        wqk_pool = ab.enter_context(tc.tile_pool(name="wqk", bufs=1))
        vpool = ab.enter_context(tc.tile_pool(name="vfull", bufs=1))
        mpool = ab.enter_context(tc.tile_pool(name="msk", bufs=1))
        qkpool = ab.enter_context(tc.tile_pool(name="qk",
                                               bufs=2 if bf16 else 1))
        epool = ab.enter_context(tc.tile_pool(name="exp",
                                              bufs=6 if bf16 else 2))
        osbpool = ab.enter_context(tc.tile_pool(name="osb", bufs=1))
        rcpool = ab.enter_context(tc.tile_pool(name="rc", bufs=1))
        # ONE shared [128,1024] (2-bank) psum pool for every paired matmul
        # group (v pairs, q+k pairs, score pairs, 1/r broadcasts): 6 banks,
        # deep enough that the exp stream never waits for a free bank.
        mmps = ab.enter_context(tc.tile_pool(name="mm_ps", bufs=3,
                                             space="PSUM"))
        ops = ab.enter_context(tc.tile_pool(name="o_ps", bufs=2,
                                            space="PSUM"))

        wq_sb = wqk_pool.tile([128, CT, F], dt_store)
        wk_sb = wqk_pool.tile([128, CT, F], dt_store)
        nc.sync.dma_start(out=wq_sb[:],
                          in_=wqT.rearrange("(ct p) f -> p ct f", p=128))
        nc.sync.dma_start(out=wk_sb[:],
                          in_=wkT.rearrange("(ct p) f -> p ct f", p=128))
        mask_sb = mpool.tile([128, 4, 512], dt_store)
        nc.sync.dma_start(out=mask_sb[:], in_=maskd[:])
        ones_sb = onepool.tile([128, 128], dt_store)
        nc.sync.dma_start(out=ones_sb[:], in_=onesd)

        # v kept as 4 quarter tiles so attention can start as soon as the
        # first quarter's projection lands (tile-granularity deps)
        v_q = [vpool.tile([128, 4, 8, D + 1], dt_store, tag=f"v{i}",
                          name=f"v_q{i}")
               for i in range(4)]
        for i in range(4):
            nc.sync.dma_start(out=v_q[i][:, :, :, D:D + 1], in_=onesd[:, 0:32])
        o_pack = opool.tile([128, HP, T], dt_store)

        def v_tile(kt, h):
            return v_q[kt // 4][:, kt % 4, h, :]

        # q/k held as per-quarter tiles, two head-pairs in flight
        q_t = [[None] * 4 for _ in range(HP)]
        k_t = [[None] * 4 for _ in range(HP)]

        def emit_qk_quarter(hp, tq, x_q):
            """Project q and k (features of head-pair hp) for one 512-token
            quarter: both 8-matmul groups land in one 2-bank psum tile."""
            fsl = slice(hp * 128, (hp + 1) * 128)
            qt = qkpool.tile([128, 512], dt_store, tag=f"q{tq}",
                             name=f"q_{hp}_{tq}")
            kt_ = qkpool.tile([128, 512], dt_store, tag=f"k{tq}",
                              name=f"k_{hp}_{tq}")
            ps = mmps.tile([128, 1024], _F32, tag="s", name=f"qkps_{hp}_{tq}")
            for ct in range(CT):
                nc.tensor.matmul(ps[:, 0:512], wq_sb[:, ct, fsl],
                                 x_q[:, ct, :],
                                 start=(ct == 0), stop=(ct == CT - 1))
            for ct in range(CT):
                nc.tensor.matmul(ps[:, 512:1024], wk_sb[:, ct, fsl],
                                 x_q[:, ct, :],
                                 start=(ct == 0), stop=(ct == CT - 1))
            nc.vector.tensor_copy(qt[:], ps[:, 0:512])
            nc.vector.tensor_copy(kt_[:], ps[:, 512:1024])
            q_t[hp][tq] = qt
            k_t[hp][tq] = kt_

        # ---- Phase A: v projection + hp0 q/k (sharing each x quarter) ----
        with tc.tile_pool(name="wv", bufs=1) as wvpool:
            wv_sb = wvpool.tile([128, CT, F], dt_store)
            nc.sync.dma_start(out=wv_sb[:],
                              in_=wvT.rearrange("(ct p) f -> p ct f", p=128))
            for tq in range(4):
                x_q = xpool.tile([128, CT, 512], dt_store, tag="x",
                                 name=f"xA{tq}")
                nc.sync.dma_start(out=x_q[:],
                                  in_=xT_r[:, :, tq * 512:(tq + 1) * 512])
                emit_qk_quarter(0, tq, x_q)
                for half in range(2):
                    ps = mmps.tile([128, 1024], _F32, tag="s",
                                   name=f"vps_{tq}_{half}")
                    for tl2 in range(2):
                        tl = half * 2 + tl2
                        for ct in range(CT):
                            nc.tensor.matmul(
                                ps[:, tl2 * 512:(tl2 + 1) * 512],
                                x_q[:, ct, tl * 128:(tl + 1) * 128],
                                wv_sb[:, ct, :],
                                start=(ct == 0), stop=(ct == CT - 1))
                    for tl2 in range(2):
                        tl = half * 2 + tl2
                        nc.vector.tensor_copy(
                            v_q[tq][:, tl, :, 0:D],
                            ps[:, tl2 * 512:(tl2 + 1) * 512].rearrange(
                                "p (h d) -> p h d", h=8))

        # ---- Phase B: attention per head-pair, next pair's q/k streamed in
        x_ahead = None
        for hp in range(HP):
            for qb in range(QB):
                if hp + 1 < HP:
                    # project the NEXT head-pair's q/k: gives the PE
                    # full-array independent work while ACT runs this
                    # pair's exps.  The x quarter is DMA'd one qb AHEAD so
                    # the projection matmuls never sit in the PE stream
                    # waiting on a 2MB transfer.
                    if qb > 0:
                        emit_qk_quarter(hp + 1, qb - 1, x_ahead)
                    x_ahead = xpool.tile([128, CT, 512], dt_store, tag="x",
                                         name=f"xB_{hp}_{qb}")
                    nc.sync.dma_start(
                        out=x_ahead[:],
                        in_=xT_r[:, :, qb * 512:(qb + 1) * 512])

                kts = 4 * (qb + 1)
                oA = ops.tile([D + 1, 512], _F32, tag="o", name=f"oA_{hp}_{qb}")
                oB = ops.tile([D + 1, 512], _F32, tag="o", name=f"oB_{hp}_{qb}")
                q_sb = q_t[hp][qb]

                pend = []  # software pipeline: scores(kt) ahead of PV(kt-1)

                def _pv(kt, e2):
                    nc.tensor.matmul(
                        oA[:], v_tile(kt, 2 * hp), e2[:, 0:512],
                        start=(kt == 0), stop=(kt == kts - 1))
                    nc.tensor.matmul(
                        oB[:], v_tile(kt, 2 * hp + 1), e2[:, 512:1024],
                        start=(kt == 0), stop=(kt == kts - 1))

                for kt in range(kts):
                    # PV of kt-1 first, then the score pair of kt, so the
                    # pair lands adjacent in the PE stream: the row-tiled
                    # pair (rows 0-63 / 64-127) runs CONCURRENTLY and
                    # covers the full array, keeping the HAM clock-gate at
                    # full speed (half-array matmuls throttle to 1.2 GHz).
                    if pend:
                        _pv(*pend.pop(0))
                    ps2 = mmps.tile([128, 1024], _F32, tag="s",
                                    name=f"sc_{hp}_{qb}_{kt}")
                    k_sb = k_t[hp][kt // 4]
                    ksl = slice((kt % 4) * 128, (kt % 4) * 128 + 128)
                    nc.tensor.matmul(ps2[:, 0:512], k_sb[0:64, ksl],
                                     q_sb[0:64, :],
                                     start=True, stop=True,
                                     tile_position=(0, 0))
                    nc.tensor.matmul(ps2[:, 512:1024], k_sb[64:128, ksl],
                                     q_sb[64:128, :],
                                     start=True, stop=True,
                                     tile_position=(64, 0))
                    e2 = epool.tile([128, 1024], dt_store, tag="e",
                                    name=f"e_{hp}_{qb}_{kt}")
                    nc.scalar.activation(e2[:], ps2[:], _EXP, scale=0.125)
                    rel = kt - 4 * qb
                    if rel >= 0:  # diagonal tile: causal mask
                        nc.vector.tensor_mul(e2[:, 0:512], e2[:, 0:512],
                                             mask_sb[:, rel, :])
                        nc.vector.tensor_mul(e2[:, 512:1024],
                                             e2[:, 512:1024],
                                             mask_sb[:, rel, :])
                    pend.append((kt, e2))
                _pv(*pend.pop(0))

                # normalize: o[d, q] * (1/r)[q];  1/r = exp(-ln(r)) on
                # ScalarE (DVE reciprocal on a 1-partition row is ~6x
                # slower), then partition-broadcast via ones-matmul.
                rc = rcpool.tile([128, 1024], _F32, tag="rc",
                                 name=f"rc_{hp}_{qb}")
                rcx = rcpool.tile([128, 1024], dt_store, tag="rcx",
                                  name=f"rcx_{hp}_{qb}")
                nc.scalar.activation(rc[64:65, 0:512], oA[64:65, :], _LN)
                nc.scalar.activation(rc[64:65, 512:1024], oB[64:65, :], _LN)
                nc.scalar.activation(rcx[64:65, :], rc[64:65, :],
                                     _EXP, scale=-1.0)
                # bc tiles reuse the (just freed) o-pool banks so the
                # shared mm pool keeps all 3 slots for the score stream
                bcA = ops.tile([128, 512], _F32, tag="o", name=f"bcA_{hp}_{qb}")
                bcB = ops.tile([128, 512], _F32, tag="o", name=f"bcB_{hp}_{qb}")
                nc.tensor.matmul(bcA, ones_sb[64:65, 0:128],
                                 rcx[64:65, 0:512],
                                 start=True, stop=True,
                                 tile_position=(64, 0))
                nc.tensor.matmul(bcB, ones_sb[64:65, 0:128],
                                 rcx[64:65, 512:1024],
                                 start=True, stop=True,
                                 tile_position=(64, 0))
                # heads live at psum partitions 0-64; head B must land at
                # o_pack partitions 64-127 -> sbuf->sbuf DMA shift
                qsl = slice(qb * 512, (qb + 1) * 512)
                tmp = osbpool.tile([64, 1024], _F32, tag="t",
                                   name=f"tmp_{hp}_{qb}")
                nc.vector.tensor_copy(tmp[0:64, 0:512], oA[0:64, :])
                nc.vector.tensor_copy(tmp[0:64, 512:1024], oB[0:64, :])
                ob2 = osbpool.tile([128, 512], _F32, tag="b",
                                   name=f"ob2_{hp}_{qb}")
                nc.sync.dma_start(out=ob2[64:128, :], in_=tmp[0:64, 512:1024])
                nc.vector.tensor_mul(o_pack[0:64, hp, qsl],
                                     tmp[0:64, 0:512], bcA[0:64, :])
                nc.vector.tensor_mul(o_pack[64:128, hp, qsl],
                                     ob2[64:128, :], bcB[64:128, :])
            if hp + 1 < HP:
                emit_qk_quarter(hp + 1, 3, x_ahead)

        ab.close()  # free A/B pools (x, weights, v, qk, exp, ...) for phase C

        # ---------------- Phase C: output projection ----------------
        with tc.tile_pool(name="wo", bufs=1) as wopool, \
             tc.tile_pool(name="y_st", bufs=4) as ystpool, \
             tc.tile_pool(name="y_ps", bufs=4, space="PSUM") as yps:
            wo_sb = wopool.tile([128, HP, C], dt_store)
            nc.sync.dma_start(out=wo_sb[:],
                              in_=woT.rearrange("(hp p) o -> p hp o", p=128))
            for ot in range(C // 128):
                for tb in range(QB):
                    ps = yps.tile([128, 512], _F32, tag="y")
                    for hp in range(HP):
                        nc.tensor.matmul(
                            ps[:], wo_sb[:, hp, ot * 128:(ot + 1) * 128],
                            o_pack[:, hp, tb * 512:(tb + 1) * 512],
                            start=(hp == 0), stop=(hp == HP - 1))
                    yst = ystpool.tile([128, 512], _F32, tag="yst")
                    nc.vector.tensor_copy(yst[:], ps[:])
                    nc.sync.dma_start(
                        out=yT[ot * 128:(ot + 1) * 128,
                               tb * 512:(tb + 1) * 512],
                        in_=yst[:])
    # The act-table pass picks the FIRST set containing each function, which
    # makes Exp and Ln thrash between two table sets (~2.7us per swap, per
    # use).  Steer both to the combined natural_log_exp_and_others set by
    # hiding exp/ln from every other set (indices into act_info.json are
    # unchanged, so act_func_set_id stays valid).
    _orig_gat = bacc.get_activation_tables

    def _gat_combined(arch):
        t = _orig_gat(arch)
        for name, fns in t.items():
            if name != "natural_log_exp_and_others":
                fns.discard(mybir.ActivationFunctionType.Exp)
                fns.discard(mybir.ActivationFunctionType.Ln)
        return t

    bacc.get_activation_tables = _gat_combined
    try:
        nc.compile()  # bacc passes: split >1-wait instrs (TRN2 ISA limit)
    finally:
        bacc.get_activation_tables = _orig_gat
    return nc


def _np_store():
    return np.float32 if DTYPE_MODE in ("f32", "f32r") else ml_dtypes.bfloat16


def _round_tf32(a):
    """Round-to-nearest-even onto the TF32 (10-bit mantissa) grid."""
    u = np.ascontiguousarray(a, dtype=np.float32).view(np.uint32)
    r = (u + 0x0FFF + ((u >> 13) & 1)) & np.uint32(0xFFFFE000)
    return r.view(np.float32)


def _prep(a):
    a = np.ascontiguousarray(a, dtype=np.float32)
    if DTYPE_MODE == "f32r":
        return _round_tf32(a)
    return a.astype(_np_store())


def _make_mask():
    kk = np.arange(128)[:, None, None]
    rr = np.arange(4)[None, :, None]
    qq = np.arange(512)[None, None, :]
    return ((rr * 128 + kk) <= qq).astype(_np_store())


LAST_RESULTS = None


def kernel(x, w_qkv, w_proj):
    global LAST_RESULTS
    if "nc" not in _cache:
        _cache["nc"] = _build_nc()
    nc = _cache["nc"]

    mask = _make_mask()
    x = np.asarray(x, dtype=np.float32).reshape(B, T, C)
    w_qkv = np.asarray(w_qkv, dtype=np.float32)
    w_proj = np.asarray(w_proj, dtype=np.float32)

    in_maps = []
    for core in range(NCORES):
        b, g = core // 2, core % 2
        fsl = slice(g * F, (g + 1) * F)
        in_maps.append({
            "xT": _prep(x[b].T),
            "wqT": _prep(w_qkv[0 * C:1 * C][fsl].T),
            "wkT": _prep(w_qkv[1 * C:2 * C][fsl].T),
            "wvT": _prep(w_qkv[2 * C:3 * C][fsl].T),
            "woT": _prep(w_proj[:, fsl].T),
            "mask": mask,
            "ones": np.ones((128, 128), _np_store()),
        })

    LAST_RESULTS = run_bass_kernel_spmd(
        nc, in_maps, list(range(NCORES)), trace=checkenv("BASS_TRACE"))

    y = np.zeros((B, T, C), np.float32)
    for core in range(NCORES):
        b = core // 2
        y[b] += LAST_RESULTS.results[core]["yT"].T
    return y
